# revision 36
# baseline (speedup 1.0000x reference)
"""MultiHeadAttention (B=2, S=2048, D=2048, H=16, RoPE) on 8 NeuronCores.

Sharding: tensor-parallel over heads. Core c owns heads 2c, 2c+1 (256 channels).
Each core: QKV projections for its channels, RoPE, full attention for its 2
heads, and a partial output projection y_c = ctx_c @ Wo[:, ch_c].T. Host sums
the 8 partials (fp16 partials, fp32 sum).

All-fp16 compute with fp32 PSUM accumulation:
  - Host pre-rearranges x and weights into partition-major layouts so every
    DMA moves partition-contiguous runs.
  - Startup: the critical tiles (x chunk 0, wv, wq/wk) are DMAed in
    kt-quarter pieces spread across BOTH rings, interleaved in first-use
    order, so the first v-sweep matmul issues ~10us in instead of ~20us
    and chunk 0 completes ~11us earlier; ~40 warmup matmuls bridge the
    preamble and warm the PE HAM clock gate.
  - Projections: per 512-token chunk, v-sweep (stationary x subtiles) then
    qk-sweep (stationary weight tiles). PSUM: q 2 + k 2 banks single-buffered
    + v 4 banks. x streams on alternating sync/gpsimd rings.
  - Attention (transposed scores): software pipeline at kt-pair granularity:
    scores+exp for item i interleave with PV matmuls for item i-1. exp runs
    on 1024-element 2-bank ACTIVATEs. The softmax denominator is built by a
    log2 reduction tree over the 16 exp tiles (DVE does the two big levels,
    the idle GpSimd engine the two small ones), leaving a single ones[128,128]
    matmul to do the partition-reduce+broadcast; its PSUM tile comes from the
    same pool as the output-projection accumulators, which frees a bank so
    ctx_ps is double-buffered: the normalize chain (den mm -> reciprocal ->
    mul) no longer blocks the next item's PV matmuls.
  - Work is ordered (b, qc, m) with m innermost so output-projection row
    blocks become available every other iteration; each iteration emits up
    to two 128-row blocks mid/late in its kt loop (their normalize chain
    finishes ~8us after queueing), keeping the PE ahead of the exp-bound
    part of the pipeline and spreading y writes evenly across the sync and
    gpsimd rings. Once scores are done, drain blocks borrow the dead score
    PSUM pool (2-bank tiles, one eviction per pair split ACT/DVE) so the
    drain is matmul-paced, each pair's half-row y DMA issued right behind
    its eviction on alternating rings; the last item's denominator tree
    stays on DVE and its normalize is split per row-block to start the
    drain sooner.
"""
import sys

sys.path.insert(0, "/opt/trn_rl_repo")

import numpy as np

B, S, D, H = 2, 2048, 2048, 16
HD = D // H          # 128
NCORES = 8
HPC = H // NCORES    # heads per core
CPC = HPC * HD       # channels per core = 256
TOK = B * S          # 4096
P = 128
KT = D // P          # 16 contraction tiles
NCH = 512            # token chunk for projections / attention qtok chunk
ROPE_BASE = 10000.0

_cache = {}


def _build_nc():
    import concourse.bass as bass  # noqa: F401
    import concourse.mybir as mybir
    import concourse.tile as tile
    from concourse import bacc
    from concourse import bass_isa

    F32 = mybir.dt.float32
    F16 = mybir.dt.float16
    AF = mybir.ActivationFunctionType
    MUL = mybir.AluOpType.mult
    ADD = mybir.AluOpType.add

    nc = bacc.Bacc(None, target_bir_lowering=False)

    NQC = TOK // NCH            # 8 projection chunks
    SQC = S // NCH              # 4 attention q-chunks per sequence
    SKT = S // P                # 16 key tiles per sequence
    VST = NCH // P              # 4 v subtiles per chunk
    QT = KT // 4                # 4 kt per x quarter tile
    HF = HD // 2                # 64
    SCALE = 1.0 / float(np.sqrt(HD))

    # host-rearranged inputs: partition-major, contiguous per partition
    xT_d = nc.dram_tensor("xR", [P, NQC, KT, NCH], F16, kind="ExternalInput")
    wq_d = nc.dram_tensor("wqR", [P, KT, CPC], F16, kind="ExternalInput")
    wk_d = nc.dram_tensor("wkR", [P, KT, CPC], F16, kind="ExternalInput")
    wv_d = nc.dram_tensor("wvR", [P, KT, CPC], F16, kind="ExternalInput")
    wo_d = nc.dram_tensor("woR", [P, HPC, D], F16, kind="ExternalInput")
    cos_d = nc.dram_tensor("cos2", [P, S], F16, kind="ExternalInput")
    sin_d = nc.dram_tensor("sin2", [P, S], F16, kind="ExternalInput")
    ones_d = nc.dram_tensor("ones128", [P, P], F16, kind="ExternalInput")
    y_d = nc.dram_tensor("y", [TOK, D], F16, kind="ExternalOutput")

    with tile.TileContext(nc) as tc, \
         nc.allow_low_precision(reason="fp16 compute, fp32 accumulate"):
        with tc.tile_pool(name="persist", bufs=1) as pp_:
            # long-lived tensors
            qT = [pp_.tile([P, TOK], F16, name=f"qT{m}") for m in range(HPC)]
            kTt = [pp_.tile([P, TOK], F16, name=f"kT{m}") for m in range(HPC)]
            vS = pp_.tile([P, TOK // P, CPC], F16, name="vS")

            wq = pp_.tile([P, KT, CPC], F16, name="wq")
            wk = pp_.tile([P, KT, CPC], F16, name="wk")
            wv = pp_.tile([P, KT, CPC], F16, name="wv")
            cos2 = pp_.tile([P, S], F16, name="cos2")
            sin2 = pp_.tile([P, S], F16, name="sin2")
            wo = pp_.tile([P, HPC, D], F16, name="wo")
            ones128 = pp_.tile([P, P], F16, name="ones128")
            def _wv(qq):
                nc.gpsimd.dma_start(
                    wv[:, qq * QT:(qq + 1) * QT, :],
                    wv_d[:, qq * QT:(qq + 1) * QT, :],
                )

            # warm the exp table set so ACT_TABLE_LOAD is off the critical path
            warm_in = pp_.tile([1, 1], F32, name="warm_in")
            warm = pp_.tile([1, 1], F32, name="warm")
            nc.vector.memset(warm_in[:], 0.0)
            nc.scalar.activation(warm[:], warm_in[:], AF.Exp)

            # a short run of dummy matmuls on a zeroed tile bridges the ~2us
            # between the preamble and the first x/wv pieces landing, and
            # starts the PE HAM warm-up window early
            wt16 = pp_.tile([P, P], F16, name="wt16")
            nc.vector.memset(wt16[:], 0.0)
            prwarm = pp_.tile([P, 1], F32, name="prwarm")
            with tc.tile_pool(name="wrm", bufs=1, space="PSUM") as wrmp:
                wrm_ps = wrmp.tile([P, P], F32, name="wrm_ps")
                for _ in range(40):
                    nc.tensor.matmul(wrm_ps[:], wt16[:], wt16[:],
                                     start=True, stop=True)

            # ---------------- Phase 1: projections + RoPE ----------------
            with tc.tile_pool(name="xp", bufs=12) as xp, \
                 tc.tile_pool(name="stg", bufs=4) as stg, \
                 tc.tile_pool(name="rp", bufs=4) as rp, \
                 tc.tile_pool(name="qkp", bufs=1, space="PSUM") as qkp, \
                 tc.tile_pool(name="vp", bufs=1, space="PSUM") as vp:
                for ch in range(NQC):
                    t0 = ch * NCH
                    s0 = (ch % SQC) * NCH  # position within sequence
                    # x for this chunk: [128, 16, 512] in four kt-quarter
                    # DMAs so chunk 0's first pieces land early; rings
                    # alternate per chunk so each carries ~half the stream
                    xt = [xp.tile([P, QT, NCH], F16, name="xt")
                          for _ in range(4)]
                    eng = nc.gpsimd if ch % 2 else nc.sync
                    if ch == 0:
                        # chunk 0 is the only DMA-paced chunk: spread its x
                        # quarters across BOTH rings, interleaved with the wv
                        # and wq/wk pieces in first-use order so neither ring
                        # serializes more than ~1MB ahead of the sweeps
                        def _x(qq, e):
                            e.dma_start(
                                xt[qq][:], xT_d[:, ch, qq * QT:(qq + 1) * QT, :]
                            )
                        def _w(qq):
                            nc.sync.dma_start(
                                wq[:, qq * QT:(qq + 1) * QT, :],
                                wq_d[:, qq * QT:(qq + 1) * QT, :],
                            )
                            nc.sync.dma_start(
                                wk[:, qq * QT:(qq + 1) * QT, :],
                                wk_d[:, qq * QT:(qq + 1) * QT, :],
                            )
                        _x(0, nc.sync)
                        _wv(0)
                        _x(1, nc.gpsimd)
                        _x(2, nc.sync)
                        _wv(1)
                        _w(0)
                        _wv(2)
                        _x(3, nc.gpsimd)
                        _w(1)
                        _wv(3)
                        _w(2)
                        _w(3)
                        nc.sync.dma_start(cos2[:], cos_d[:])
                        nc.sync.dma_start(sin2[:], sin_d[:])
                    else:
                        for qq in range(4):
                            eng.dma_start(
                                xt[qq][:], xT_d[:, ch, qq * QT:(qq + 1) * QT, :]
                            )
                    if ch == 1:
                        # needed only in phase 2; behind chunk 1 on gpsimd
                        nc.gpsimd.dma_start(wo[:], wo_d[:])
                        nc.gpsimd.dma_start(ones128[:], ones_d[:])
                    if ch == 6:
                        # warm partition_all_reduce's ~6us Q7 IRAM load here
                        # (gpsimd's remaining phase-1 work is far ahead); the
                        # real calls replace the PE denominator matmul
                        nc.gpsimd.partition_all_reduce(
                            prwarm[:], wt16[:, 0:1], P,
                            bass_isa.ReduceOp.add,
                        )

                    # ---- v-sweep: stationary x subtiles, moving wv ----
                    v_ps = vp.tile([P, VST, 512], F32, name="v_ps")
                    for kt in range(KT):
                        xtile = xt[kt // QT][:, kt % QT, :]
                        st_, sp_ = (kt == 0), (kt == KT - 1)
                        for st in range(VST):
                            nc.tensor.matmul(
                                v_ps[:, st, 0:CPC],
                                xtile[:, st * P:(st + 1) * P],
                                wv[:, kt, :],
                                start=st_, stop=sp_,
                            )
                    # evict v on ScalarE (fp32 psum -> fp16 sbuf)
                    nc.scalar.activation(
                        vS[:, ch * VST:(ch + 1) * VST, :], v_ps[:, :, 0:CPC],
                        AF.Copy,
                    )

                    # ---- qk-sweep: stationary weights, moving x ----
                    q_ps = qkp.tile([P, HPC, NCH], F32, name="q_ps")
                    k_ps = qkp.tile([P, HPC, NCH], F32, name="k_ps")
                    for kt in range(KT):
                        xtile = xt[kt // QT][:, kt % QT, :]
                        st_, sp_ = (kt == 0), (kt == KT - 1)
                        for m in range(HPC):
                            nc.tensor.matmul(
                                q_ps[:, m, :], wq[:, kt, m * P:(m + 1) * P],
                                xtile[:], start=st_, stop=sp_,
                            )
                            nc.tensor.matmul(
                                k_ps[:, m, :], wk[:, kt, m * P:(m + 1) * P],
                                xtile[:], start=st_, stop=sp_,
                            )
                    # stage q/k to SBUF fp16 on ScalarE (frees PSUM fast)
                    qsb = stg.tile([P, HPC, NCH], F16, name="qsb")
                    ksb = stg.tile([P, HPC, NCH], F16, name="ksb")
                    nc.scalar.activation(qsb[:], q_ps[:], AF.Copy)
                    nc.scalar.activation(ksb[:], k_ps[:], AF.Copy)

                    # RoPE on VectorE in fp16: out = src*cos2 + swap(src)*sin2
                    # sin2 is laid out [s; -s] so each half-product reads its
                    # inputs at a shared base partition (DVE requirement) and
                    # only the OUTPUT lands in the opposite half:
                    #   rot[0:64]   = src[64:128]*sin2[64:128]  (= -s half)
                    #   rot[64:128] = src[0:64]  *sin2[0:64]    (= +s half)
                    for m in range(HPC):
                        for src, dst in ((qsb, qT[m]), (ksb, kTt[m])):
                            sm = src[:, m, :]
                            rot = rp.tile([P, NCH], F16, name="rot")
                            nc.vector.tensor_tensor(
                                rot[0:HF, :], sm[HF:P, :],
                                sin2[HF:P, s0:s0 + NCH], MUL,
                            )
                            nc.vector.tensor_tensor(
                                rot[HF:P, :], sm[0:HF, :],
                                sin2[0:HF, s0:s0 + NCH], MUL,
                            )
                            tmp = rp.tile([P, NCH], F16, name="tmp")
                            nc.vector.tensor_tensor(
                                tmp[:], sm[:], cos2[:, s0:s0 + NCH], MUL
                            )
                            nc.vector.tensor_tensor(
                                dst[:, t0:t0 + NCH], tmp[:], rot[:], ADD
                            )

            # ---------------- Phase 2+3: attention + output projection ----
            with tc.tile_pool(name="ep", bufs=2) as ep, \
                 tc.tile_pool(name="tp", bufs=2) as tp, \
                 tc.tile_pool(name="dp", bufs=2) as dp, \
                 tc.tile_pool(name="yp", bufs=4) as yp, \
                 tc.tile_pool(name="ctxp", bufs=1) as ctxp, \
                 tc.tile_pool(name="cp", bufs=2, space="PSUM") as cpsum, \
                 tc.tile_pool(name="ap", bufs=2, space="PSUM") as apsum, \
                 tc.tile_pool(name="sp2", bufs=2, space="PSUM") as spsum:
                ctxT = [
                    ctxp.tile([P, S], F16, name=f"ctxT{b}_{m}")
                    for b in range(B)
                    for m in range(HPC)
                ]

                nblk = [0]  # global oproj block counter (ring + evict split)

                def oproj_block(b, tt, tail=False):
                    # one 128-row block of y: 4 nck groups x 2 heads
                    row0 = b * S + tt * P
                    i = nblk[0]
                    nblk[0] += 1
                    y_sb = yp.tile([P, D], F16, name="y_sb")
                    if tail:
                        # scores are finished: borrow the (dead) scr PSUM
                        # pool -- 2-bank tiles, one eviction per pair and a
                        # half-row DMA right behind it, so the drain is
                        # matmul-paced and the HBM writes start ASAP
                        for pr in range(2):
                            y2 = spsum.tile([P, 2, NCH], F32, name="scr")
                            for j in range(2):
                                nck = pr * 2 + j
                                for m in range(HPC):
                                    nc.tensor.matmul(
                                        y2[:, j, :],
                                        ctxT[b * HPC + m][:, tt * P:(tt + 1) * P],
                                        wo[:, m, nck * NCH:(nck + 1) * NCH],
                                        start=(m == 0), stop=(m == HPC - 1),
                                    )
                            half = y_sb[:, pr * (D // 2):(pr + 1) * (D // 2)]
                            if pr == 0:
                                nc.scalar.activation(half, y2[:], AF.Copy)
                            else:
                                nc.vector.tensor_copy(half, y2[:])
                            eng = nc.sync if (i + pr) % 2 == 0 else nc.gpsimd
                            eng.dma_start(
                                y_d[row0:row0 + P,
                                    pr * (D // 2):(pr + 1) * (D // 2)],
                                half,
                            )
                        return
                    for nck in range(D // NCH):
                        y_ps = apsum.tile([P, NCH], F32, name="y_ps")
                        for m in range(HPC):
                            nc.tensor.matmul(
                                y_ps[:],
                                ctxT[b * HPC + m][:, tt * P:(tt + 1) * P],
                                wo[:, m, nck * NCH:(nck + 1) * NCH],
                                start=(m == 0), stop=(m == HPC - 1),
                            )
                        # ~1 in 8 evictions on ScalarE (it is exp-bound),
                        # rest on VectorE
                        if (i * 4 + nck) % 8 == 3:
                            nc.scalar.activation(
                                y_sb[:, nck * NCH:(nck + 1) * NCH],
                                y_ps[:], AF.Copy,
                            )
                        else:
                            nc.vector.tensor_copy(
                                y_sb[:, nck * NCH:(nck + 1) * NCH], y_ps[:],
                            )
                    eng = nc.sync if i % 2 == 0 else nc.gpsimd
                    eng.dma_start(y_d[row0:row0 + P, :], y_sb[:])

                def fused_iter(cur, prev, ex_prev, oproj_q):
                    """Interleave scores+exp for `cur` with PV matmuls for
                    `prev` at kt-pair granularity; emit up to two queued
                    output-projection blocks per iteration."""
                    blks = [oproj_q.pop(0) for _ in
                            range(min(2, len(oproj_q)))]
                    ex_new = None
                    if cur is not None:
                        bC, mC, qcC = cur
                        qt0 = bC * S + qcC * NCH
                        ex_new = ep.tile([P, SKT, NCH], F16, name="ex")
                    if prev is not None:
                        bP, mP, qcP = prev
                        ctx_ps = cpsum.tile([P, NCH], F32, name="ctx_ps")
                        # denominator reduction tree over ex_prev's 16 tiles:
                        # two big levels on DVE, two small ones on GpSimd
                        # (idle in this phase); a single ones-matmul then
                        # partition-reduces + broadcasts
                        t8 = tp.tile([P, 8, NCH], F16, name="t8")
                        u4 = tp.tile([P, 4, NCH], F16, name="u4")
                        es = tp.tile([P, 2, NCH], F16, name="es")
                        nc.vector.tensor_tensor(
                            t8[:], ex_prev[:, 0:8, :], ex_prev[:, 8:16, :],
                            ADD,
                        )
                        nc.vector.tensor_tensor(
                            u4[:], t8[:, 0:4, :], t8[:, 4:8, :], ADD,
                        )
                        # the small tree levels go to the idle GpSimd engine
                        # (relieves DVE), except for the final item where the
                        # extra cross-engine latency would push the drain out
                        tree_eng = nc.vector if cur is None else nc.gpsimd
                        tree_eng.tensor_tensor(
                            es[:], u4[:, 0:2, :], u4[:, 2:4, :], ADD,
                        )
                        tree_eng.tensor_tensor(
                            es[:, 0, :], es[:, 0, :], es[:, 1, :], ADD,
                        )
                    for kp in range(SKT // 2):
                        # oproj blocks sit mid/late in the iteration: their
                        # (b, qc) normalize chain (tree -> den -> recip ->
                        # mul) completes ~8us after the queueing iteration
                        # starts, so an iteration-start emission would stall
                        if kp == 4 and blks:
                            oproj_block(*blks[0], tail=(cur is None))
                        if kp == 7 and len(blks) > 1:
                            oproj_block(*blks[1], tail=(cur is None))
                        if cur is not None:
                            scr = spsum.tile([P, 2, NCH], F32, name="scr")
                            for j in range(2):
                                kt = kp * 2 + j
                                nc.tensor.matmul(
                                    scr[:, j, :],
                                    kTt[mC][:, bC * S + kt * P:
                                             bC * S + (kt + 1) * P],
                                    qT[mC][:, qt0:qt0 + NCH],
                                    start=True, stop=True,
                                )
                            nc.scalar.activation(
                                ex_new[:, kp * 2:kp * 2 + 2, :], scr[:],
                                AF.Exp, scale=SCALE,
                            )
                        if prev is not None:
                            for j in range(2):
                                kt = kp * 2 + j
                                gkt = bP * SKT + kt
                                st_, sp_ = (kt == 0), (kt == SKT - 1)
                                nc.tensor.matmul(
                                    ctx_ps[:],
                                    vS[:, gkt, mP * P:(mP + 1) * P],
                                    ex_prev[:, kt, :],
                                    start=st_, stop=sp_,
                                )

                    if prev is not None:
                        if cur is None:
                            # final item: ones-matmul partition-reduce (short
                            # latency protects the drain); den_ps shares the
                            # oproj PSUM pool
                            den_ps = apsum.tile([P, NCH], F32, name="y_ps")
                            nc.tensor.matmul(
                                den_ps[:], ones128[:], es[:, 0, :],
                                start=True, stop=True,
                            )
                            rec_src = den_ps
                        else:
                            # steady state: partition-reduce + broadcast on
                            # the idle GpSimd daisy chain, saving the PE a
                            # matmul per item (fp32 internal accumulation)
                            den_sb = dp.tile([P, NCH], F32, name="den_sb")
                            nc.gpsimd.partition_all_reduce(
                                den_sb[:], es[:, 0, :], P,
                                bass_isa.ReduceOp.add,
                            )
                            rec_src = den_sb
                        rec = dp.tile([P, NCH], F32, name="rec")
                        nc.vector.reciprocal_approx_fast(out=rec[:], in_=rec_src[:])
                        if cur is None:
                            # final item: normalize per row-block so each
                            # drain oproj block starts behind its own slice
                            for j in range(VST):
                                sl = slice(j * P, (j + 1) * P)
                                nc.vector.tensor_tensor(
                                    ctxT[bP * HPC + mP][
                                        :, qcP * NCH + j * P:
                                        qcP * NCH + (j + 1) * P],
                                    ctx_ps[:, sl], rec[:, sl], MUL,
                                )
                        else:
                            nc.vector.tensor_tensor(
                                ctxT[bP * HPC + mP][:, qcP * NCH:(qcP + 1) * NCH],
                                ctx_ps[:], rec[:], MUL,
                            )
                        if mP == HPC - 1:
                            oproj_q.extend(
                                (bP, qcP * VST + j) for j in range(VST))
                    return ex_new

                # m innermost so each (b, qc)'s output rows unblock every
                # other iteration -> oproj work and y DMAs spread evenly
                work = [(b, m, qc) for b in range(B) for qc in range(SQC)
                        for m in range(HPC)]
                ex_prev = None
                oproj_q = []
                for i in range(len(work) + 1):
                    cur = work[i] if i < len(work) else None
                    prev = work[i - 1] if i > 0 else None
                    ex_prev = fused_iter(cur, prev, ex_prev, oproj_q)
                while oproj_q:
                    oproj_block(*oproj_q.pop(0), tail=True)
    nc.finalize()
    return nc


def _rope_tables():
    inv_freq = (1.0 / (ROPE_BASE ** (np.arange(0, HD, 2, dtype=np.float32) / HD))).astype(np.float32)
    t = np.arange(S, dtype=np.float32)
    freqs = np.outer(t, inv_freq).astype(np.float32)  # [S, HD/2]
    c = np.cos(freqs).astype(np.float32).T            # [64, S]
    s = np.sin(freqs).astype(np.float32).T
    cos2 = np.concatenate([c, c], axis=0)             # [128, S]
    sin2 = np.concatenate([s, -s], axis=0)            # [128, S]: [s; -s]
    return np.ascontiguousarray(cos2), np.ascontiguousarray(sin2)


def kernel(x, Wq, Wk, Wv, Wo):
    from concourse.bass_utils import run_bass_kernel_spmd

    F16 = np.float16
    NQC = TOK // NCH

    x = np.asarray(x, dtype=np.float32)
    Wq = np.asarray(Wq, dtype=np.float32)
    Wk = np.asarray(Wk, dtype=np.float32)
    Wv = np.asarray(Wv, dtype=np.float32)
    Wo = np.asarray(Wo, dtype=np.float32)

    # x rearranged to [p, chunk, kt, token]: partition-contiguous DMA runs
    xR = np.ascontiguousarray(
        x.reshape(NQC, NCH, KT, P).transpose(3, 0, 2, 1).astype(F16)
    )
    cos2, sin2 = _rope_tables()
    cos2 = np.ascontiguousarray(cos2.astype(F16))
    sin2 = np.ascontiguousarray(sin2.astype(F16))
    ones128 = np.ones((P, P), dtype=F16)

    def wslices(W, c):  # [D, CPC] -> [p, kt, cpc]
        ch0, ch1 = c * CPC, (c + 1) * CPC
        wT = W[ch0:ch1, :].T  # [D, CPC]
        return np.ascontiguousarray(
            wT.reshape(KT, P, CPC).transpose(1, 0, 2).astype(F16)
        )

    in_maps = []
    for c in range(NCORES):
        ch0, ch1 = c * CPC, (c + 1) * CPC
        woT = Wo[:, ch0:ch1].T  # [CPC, D]
        in_maps.append({
            "xR": xR,
            "wqR": wslices(Wq, c),
            "wkR": wslices(Wk, c),
            "wvR": wslices(Wv, c),
            "woR": np.ascontiguousarray(
                woT.reshape(HPC, P, D).transpose(1, 0, 2).astype(F16)
            ),
            "cos2": cos2,
            "sin2": sin2,
            "ones128": ones128,
        })

    if "nc" not in _cache:
        _cache["nc"] = _build_nc()
    res = run_bass_kernel_spmd(_cache["nc"], in_maps, core_ids=list(range(NCORES)))
    _cache["last_results"] = res

    y = np.zeros((TOK, D), dtype=np.float32)
    for rm in res.results:
        y += rm["y"].astype(np.float32)
    return y.reshape(B, S, D)


# revision 39
# speedup vs baseline: 1.5581x; 1.5581x over previous
"""MultiHeadAttention (B=2, S=2048, D=2048, H=16, RoPE) on 8 NeuronCores.

Sharding: tensor-parallel over heads. Core c owns heads 2c, 2c+1 (256 channels).
Each core: QKV projections for its channels, RoPE, full attention for its 2
heads, and a partial output projection y_c = ctx_c @ Wo[:, ch_c].T. Host sums
the 8 partials (fp16 partials, fp32 sum).

All-fp16 compute with fp32 PSUM accumulation:
  - Host pre-rearranges x and weights into partition-major layouts so every
    DMA moves partition-contiguous runs.
  - Startup: the critical tiles (x chunk 0, wv, wq/wk) are DMAed in
    kt-quarter pieces spread across BOTH rings, interleaved in first-use
    order, so the first v-sweep matmul issues ~10us in instead of ~20us
    and chunk 0 completes ~11us earlier; ~40 warmup matmuls bridge the
    preamble and warm the PE HAM clock gate.
  - Projections: per 512-token chunk, v-sweep (stationary x subtiles) then
    qk-sweep (stationary weight tiles). PSUM: q 2 + k 2 banks single-buffered
    + v 4 banks. x streams on alternating sync/gpsimd rings.
  - Attention (transposed scores): software pipeline at kt-pair granularity:
    scores+exp for item i interleave with PV matmuls for item i-1. exp runs
    on 1024-element 2-bank ACTIVATEs. The softmax denominator is built by a
    log2 reduction tree over the 16 exp tiles (DVE does the two big levels,
    the idle GpSimd engine the two small ones), leaving a single ones[128,128]
    matmul to do the partition-reduce+broadcast; its PSUM tile comes from the
    same pool as the output-projection accumulators, which frees a bank so
    ctx_ps is double-buffered: the normalize chain (den mm -> reciprocal ->
    mul) no longer blocks the next item's PV matmuls.
  - Work is ordered (b, qc, m) with m innermost so output-projection row
    blocks become available every other iteration; each iteration emits up
    to two 128-row blocks mid/late in its kt loop (their normalize chain
    finishes ~8us after queueing), keeping the PE ahead of the exp-bound
    part of the pipeline and spreading y writes evenly across the sync and
    gpsimd rings. Once scores are done, drain blocks borrow the dead score
    PSUM pool (2-bank tiles, one eviction per pair split ACT/DVE) so the
    drain is matmul-paced, each pair's half-row y DMA issued right behind
    its eviction on alternating rings; the last item's denominator tree
    stays on DVE and its normalize is split per row-block to start the
    drain sooner.
"""
import sys

sys.path.insert(0, "/opt/trn_rl_repo")

import numpy as np

B, S, D, H = 2, 2048, 2048, 16
HD = D // H          # 128
NCORES = 8
HPC = H // NCORES    # heads per core
CPC = HPC * HD       # channels per core = 256
TOK = B * S          # 4096
P = 128
KT = D // P          # 16 contraction tiles
NCH = 512            # token chunk for projections / attention qtok chunk
ROPE_BASE = 10000.0

_cache = {}


def _build_nc():
    import concourse.bass as bass  # noqa: F401
    import concourse.mybir as mybir
    import concourse.tile as tile
    from concourse import bacc

    F32 = mybir.dt.float32
    F16 = mybir.dt.float16
    AF = mybir.ActivationFunctionType
    MUL = mybir.AluOpType.mult
    ADD = mybir.AluOpType.add

    nc = bacc.Bacc(None, target_bir_lowering=False)

    NQC = TOK // NCH            # 8 projection chunks
    SQC = S // NCH              # 4 attention q-chunks per sequence
    SKT = S // P                # 16 key tiles per sequence
    VST = NCH // P              # 4 v subtiles per chunk
    QT = KT // 4                # 4 kt per x quarter tile
    HF = HD // 2                # 64
    SCALE = 1.0 / float(np.sqrt(HD))

    # host-rearranged inputs: partition-major, contiguous per partition
    xT_d = nc.dram_tensor("xR", [P, NQC, KT, NCH], F16, kind="ExternalInput")
    wq_d = nc.dram_tensor("wqR", [P, KT, CPC], F16, kind="ExternalInput")
    wk_d = nc.dram_tensor("wkR", [P, KT, CPC], F16, kind="ExternalInput")
    wv_d = nc.dram_tensor("wvR", [P, KT, CPC], F16, kind="ExternalInput")
    wo_d = nc.dram_tensor("woR", [P, HPC, D], F16, kind="ExternalInput")
    cos_d = nc.dram_tensor("cos2", [P, S], F16, kind="ExternalInput")
    sin_d = nc.dram_tensor("sin2", [P, S], F16, kind="ExternalInput")
    ones_d = nc.dram_tensor("ones128", [P, P], F16, kind="ExternalInput")
    y_d = nc.dram_tensor("y", [TOK, D], F16, kind="ExternalOutput")

    with tile.TileContext(nc) as tc, \
         nc.allow_low_precision(reason="fp16 compute, fp32 accumulate"):
        with tc.tile_pool(name="persist", bufs=1) as pp_:
            # long-lived tensors
            qT = [pp_.tile([P, TOK], F16, name=f"qT{m}") for m in range(HPC)]
            kTt = [pp_.tile([P, TOK], F16, name=f"kT{m}") for m in range(HPC)]
            vS = pp_.tile([P, TOK // P, CPC], F16, name="vS")

            wq = pp_.tile([P, KT, CPC], F16, name="wq")
            wk = pp_.tile([P, KT, CPC], F16, name="wk")
            wv = pp_.tile([P, KT, CPC], F16, name="wv")
            cos2 = pp_.tile([P, S], F16, name="cos2")
            sin2 = pp_.tile([P, S], F16, name="sin2")
            wo = pp_.tile([P, HPC, D], F16, name="wo")
            ones128 = pp_.tile([P, P], F16, name="ones128")
            def _wv(qq, e):
                e.dma_start(
                    wv[:, qq * QT:(qq + 1) * QT, :],
                    wv_d[:, qq * QT:(qq + 1) * QT, :],
                )

            # warm the exp table set so ACT_TABLE_LOAD is off the critical path
            warm_in = pp_.tile([1, 1], F32, name="warm_in")
            warm = pp_.tile([1, 1], F32, name="warm")
            nc.vector.memset(warm_in[:], 0.0)
            nc.scalar.activation(warm[:], warm_in[:], AF.Exp)

            # a short run of dummy matmuls on a zeroed tile bridges the ~2us
            # between the preamble and the first x/wv pieces landing, and
            # starts the PE HAM warm-up window early
            wt16 = pp_.tile([P, P], F16, name="wt16")
            nc.vector.memset(wt16[:], 0.0)
            with tc.tile_pool(name="wrm", bufs=1, space="PSUM") as wrmp:
                wrm_ps = wrmp.tile([P, P], F32, name="wrm_ps")
                for _ in range(40):
                    nc.tensor.matmul(wrm_ps[:], wt16[:], wt16[:],
                                     start=True, stop=True)

            # ---------------- Phase 1: projections + RoPE ----------------
            with tc.tile_pool(name="xp", bufs=12) as xp, \
                 tc.tile_pool(name="stg", bufs=4) as stg, \
                 tc.tile_pool(name="rp", bufs=4) as rp, \
                 tc.tile_pool(name="qkp", bufs=1, space="PSUM") as qkp, \
                 tc.tile_pool(name="vp", bufs=1, space="PSUM") as vp:
                for ch in range(NQC):
                    t0 = ch * NCH
                    s0 = (ch % SQC) * NCH  # position within sequence
                    # x for this chunk: [128, 16, 512] in four kt-quarter
                    # DMAs so chunk 0's first pieces land early; rings
                    # alternate per chunk so each carries ~half the stream
                    xt = [xp.tile([P, QT, NCH], F16, name="xt")
                          for _ in range(4)]
                    eng = nc.gpsimd if ch % 2 else nc.sync
                    if ch == 0:
                        # chunk 0 is the only DMA-paced chunk: spread its x
                        # quarters across BOTH rings, interleaved with the wv
                        # and wq/wk pieces in first-use order so neither ring
                        # serializes more than ~1MB ahead of the sweeps
                        def _x(qq, e):
                            e.dma_start(
                                xt[qq][:], xT_d[:, ch, qq * QT:(qq + 1) * QT, :]
                            )
                        def _w(qq):
                            nc.sync.dma_start(
                                wq[:, qq * QT:(qq + 1) * QT, :],
                                wq_d[:, qq * QT:(qq + 1) * QT, :],
                            )
                            nc.sync.dma_start(
                                wk[:, qq * QT:(qq + 1) * QT, :],
                                wk_d[:, qq * QT:(qq + 1) * QT, :],
                            )
                        # wv quarters alternate rings too, so each ring
                        # delivers exactly one (x, wv) kt-quarter pair per
                        # v-sweep consumption window -- no single ring holds
                        # the whole wv stream behind other traffic
                        _x(0, nc.sync)
                        _x(1, nc.gpsimd)
                        _wv(0, nc.sync)
                        _wv(1, nc.gpsimd)
                        _x(2, nc.sync)
                        _x(3, nc.gpsimd)
                        _wv(2, nc.sync)
                        _wv(3, nc.gpsimd)
                        _w(0)
                        _w(1)
                        _w(2)
                        _w(3)
                        nc.sync.dma_start(cos2[:], cos_d[:])
                        nc.sync.dma_start(sin2[:], sin_d[:])
                    else:
                        for qq in range(4):
                            eng.dma_start(
                                xt[qq][:], xT_d[:, ch, qq * QT:(qq + 1) * QT, :]
                            )
                    if ch == 1:
                        # needed only in phase 2; behind chunk 1 on gpsimd
                        nc.gpsimd.dma_start(wo[:], wo_d[:])
                        nc.gpsimd.dma_start(ones128[:], ones_d[:])

                    # ---- v-sweep: stationary x subtiles, moving wv ----
                    v_ps = vp.tile([P, VST, 512], F32, name="v_ps")
                    for kt in range(KT):
                        xtile = xt[kt // QT][:, kt % QT, :]
                        st_, sp_ = (kt == 0), (kt == KT - 1)
                        for st in range(VST):
                            nc.tensor.matmul(
                                v_ps[:, st, 0:CPC],
                                xtile[:, st * P:(st + 1) * P],
                                wv[:, kt, :],
                                start=st_, stop=sp_,
                            )
                    # evict v on ScalarE (fp32 psum -> fp16 sbuf)
                    nc.scalar.activation(
                        vS[:, ch * VST:(ch + 1) * VST, :], v_ps[:, :, 0:CPC],
                        AF.Copy,
                    )

                    # ---- qk-sweep: stationary weights, moving x ----
                    q_ps = qkp.tile([P, HPC, NCH], F32, name="q_ps")
                    k_ps = qkp.tile([P, HPC, NCH], F32, name="k_ps")
                    for kt in range(KT):
                        xtile = xt[kt // QT][:, kt % QT, :]
                        st_, sp_ = (kt == 0), (kt == KT - 1)
                        for m in range(HPC):
                            nc.tensor.matmul(
                                q_ps[:, m, :], wq[:, kt, m * P:(m + 1) * P],
                                xtile[:], start=st_, stop=sp_,
                            )
                            nc.tensor.matmul(
                                k_ps[:, m, :], wk[:, kt, m * P:(m + 1) * P],
                                xtile[:], start=st_, stop=sp_,
                            )
                    # stage q/k to SBUF fp16 on ScalarE (frees PSUM fast)
                    qsb = stg.tile([P, HPC, NCH], F16, name="qsb")
                    ksb = stg.tile([P, HPC, NCH], F16, name="ksb")
                    nc.scalar.activation(qsb[:], q_ps[:], AF.Copy)
                    nc.scalar.activation(ksb[:], k_ps[:], AF.Copy)

                    # RoPE on VectorE in fp16: out = src*cos2 + swap(src)*sin2
                    # sin2 is laid out [s; -s] so each half-product reads its
                    # inputs at a shared base partition (DVE requirement) and
                    # only the OUTPUT lands in the opposite half:
                    #   rot[0:64]   = src[64:128]*sin2[64:128]  (= -s half)
                    #   rot[64:128] = src[0:64]  *sin2[0:64]    (= +s half)
                    for m in range(HPC):
                        for src, dst in ((qsb, qT[m]), (ksb, kTt[m])):
                            sm = src[:, m, :]
                            rot = rp.tile([P, NCH], F16, name="rot")
                            nc.vector.tensor_tensor(
                                rot[0:HF, :], sm[HF:P, :],
                                sin2[HF:P, s0:s0 + NCH], MUL,
                            )
                            nc.vector.tensor_tensor(
                                rot[HF:P, :], sm[0:HF, :],
                                sin2[0:HF, s0:s0 + NCH], MUL,
                            )
                            tmp = rp.tile([P, NCH], F16, name="tmp")
                            nc.vector.tensor_tensor(
                                tmp[:], sm[:], cos2[:, s0:s0 + NCH], MUL
                            )
                            nc.vector.tensor_tensor(
                                dst[:, t0:t0 + NCH], tmp[:], rot[:], ADD
                            )

            # ---------------- Phase 2+3: attention + output projection ----
            with tc.tile_pool(name="ep", bufs=2) as ep, \
                 tc.tile_pool(name="tp", bufs=2) as tp, \
                 tc.tile_pool(name="dp", bufs=2) as dp, \
                 tc.tile_pool(name="yp", bufs=4) as yp, \
                 tc.tile_pool(name="ctxp", bufs=1) as ctxp, \
                 tc.tile_pool(name="cp", bufs=2, space="PSUM") as cpsum, \
                 tc.tile_pool(name="ap", bufs=2, space="PSUM") as apsum, \
                 tc.tile_pool(name="sp2", bufs=2, space="PSUM") as spsum:
                ctxT = [
                    ctxp.tile([P, S], F16, name=f"ctxT{b}_{m}")
                    for b in range(B)
                    for m in range(HPC)
                ]

                nblk = [0]  # global oproj block counter (ring + evict split)

                def oproj_block(b, tt, tail=False):
                    # one 128-row block of y: 4 nck groups x 2 heads
                    row0 = b * S + tt * P
                    i = nblk[0]
                    nblk[0] += 1
                    y_sb = yp.tile([P, D], F16, name="y_sb")
                    if tail:
                        # scores are finished: borrow the (dead) scr PSUM
                        # pool -- 2-bank tiles, one eviction per pair and a
                        # half-row DMA right behind it, so the drain is
                        # matmul-paced and the HBM writes start ASAP
                        for pr in range(2):
                            y2 = spsum.tile([P, 2, NCH], F32, name="scr")
                            for j in range(2):
                                nck = pr * 2 + j
                                for m in range(HPC):
                                    nc.tensor.matmul(
                                        y2[:, j, :],
                                        ctxT[b * HPC + m][:, tt * P:(tt + 1) * P],
                                        wo[:, m, nck * NCH:(nck + 1) * NCH],
                                        start=(m == 0), stop=(m == HPC - 1),
                                    )
                            half = y_sb[:, pr * (D // 2):(pr + 1) * (D // 2)]
                            if pr == 0:
                                nc.scalar.activation(half, y2[:], AF.Copy)
                            else:
                                nc.vector.tensor_copy(half, y2[:])
                            eng = nc.sync if (i + pr) % 2 == 0 else nc.gpsimd
                            eng.dma_start(
                                y_d[row0:row0 + P,
                                    pr * (D // 2):(pr + 1) * (D // 2)],
                                half,
                            )
                        return
                    for nck in range(D // NCH):
                        y_ps = apsum.tile([P, NCH], F32, name="y_ps")
                        for m in range(HPC):
                            nc.tensor.matmul(
                                y_ps[:],
                                ctxT[b * HPC + m][:, tt * P:(tt + 1) * P],
                                wo[:, m, nck * NCH:(nck + 1) * NCH],
                                start=(m == 0), stop=(m == HPC - 1),
                            )
                        # ~1 in 8 evictions on ScalarE (it is exp-bound),
                        # rest on VectorE
                        if (i * 4 + nck) % 8 == 3:
                            nc.scalar.activation(
                                y_sb[:, nck * NCH:(nck + 1) * NCH],
                                y_ps[:], AF.Copy,
                            )
                        else:
                            nc.vector.tensor_copy(
                                y_sb[:, nck * NCH:(nck + 1) * NCH], y_ps[:],
                            )
                    eng = nc.sync if i % 2 == 0 else nc.gpsimd
                    eng.dma_start(y_d[row0:row0 + P, :], y_sb[:])

                def fused_iter(cur, prev, ex_prev, oproj_q):
                    """Interleave scores+exp for `cur` with PV matmuls for
                    `prev` at kt-pair granularity; emit up to two queued
                    output-projection blocks per iteration."""
                    blks = [oproj_q.pop(0) for _ in
                            range(min(2, len(oproj_q)))]
                    ex_new = None
                    if cur is not None:
                        bC, mC, qcC = cur
                        qt0 = bC * S + qcC * NCH
                        ex_new = ep.tile([P, SKT, NCH], F16, name="ex")
                    if prev is not None:
                        bP, mP, qcP = prev
                        ctx_ps = cpsum.tile([P, NCH], F32, name="ctx_ps")
                        # denominator reduction tree over ex_prev's 16 tiles:
                        # two big levels on DVE, two small ones on GpSimd
                        # (idle in this phase); a single ones-matmul then
                        # partition-reduces + broadcasts
                        t8 = tp.tile([P, 8, NCH], F16, name="t8")
                        u4 = tp.tile([P, 4, NCH], F16, name="u4")
                        es = tp.tile([P, 2, NCH], F16, name="es")
                        nc.vector.tensor_tensor(
                            t8[:], ex_prev[:, 0:8, :], ex_prev[:, 8:16, :],
                            ADD,
                        )
                        nc.vector.tensor_tensor(
                            u4[:], t8[:, 0:4, :], t8[:, 4:8, :], ADD,
                        )
                        # the small tree levels go to the idle GpSimd engine
                        # (relieves DVE), except for the final item where the
                        # extra cross-engine latency would push the drain out
                        tree_eng = nc.vector if cur is None else nc.gpsimd
                        tree_eng.tensor_tensor(
                            es[:], u4[:, 0:2, :], u4[:, 2:4, :], ADD,
                        )
                        tree_eng.tensor_tensor(
                            es[:, 0, :], es[:, 0, :], es[:, 1, :], ADD,
                        )
                    for kp in range(SKT // 2):
                        # oproj blocks sit mid/late in the iteration: their
                        # (b, qc) normalize chain (tree -> den -> recip ->
                        # mul) completes ~8us after the queueing iteration
                        # starts, so an iteration-start emission would stall
                        if kp == 4 and blks:
                            oproj_block(*blks[0], tail=(cur is None))
                        if kp == 7 and len(blks) > 1:
                            oproj_block(*blks[1], tail=(cur is None))
                        if cur is not None:
                            scr = spsum.tile([P, 2, NCH], F32, name="scr")
                            for j in range(2):
                                kt = kp * 2 + j
                                nc.tensor.matmul(
                                    scr[:, j, :],
                                    kTt[mC][:, bC * S + kt * P:
                                             bC * S + (kt + 1) * P],
                                    qT[mC][:, qt0:qt0 + NCH],
                                    start=True, stop=True,
                                )
                            nc.scalar.activation(
                                ex_new[:, kp * 2:kp * 2 + 2, :], scr[:],
                                AF.Exp, scale=SCALE,
                            )
                        if prev is not None:
                            for j in range(2):
                                kt = kp * 2 + j
                                gkt = bP * SKT + kt
                                st_, sp_ = (kt == 0), (kt == SKT - 1)
                                nc.tensor.matmul(
                                    ctx_ps[:],
                                    vS[:, gkt, mP * P:(mP + 1) * P],
                                    ex_prev[:, kt, :],
                                    start=st_, stop=sp_,
                                )

                    if prev is not None:
                        # den_ps shares the oproj PSUM pool (frees a bank so
                        # ctx_ps is double-buffered); every row of the
                        # full-stationary accumulator IS the denominator
                        den_ps = apsum.tile([P, NCH], F32, name="y_ps")
                        nc.tensor.matmul(
                            den_ps[:], ones128[:], es[:, 0, :],
                            start=True, stop=True,
                        )
                        rec = dp.tile([P, NCH], F32, name="rec")
                        nc.vector.reciprocal_approx_fast(out=rec[:], in_=den_ps[:])
                        if cur is None:
                            # final item: normalize per row-block so each
                            # drain oproj block starts behind its own slice
                            for j in range(VST):
                                sl = slice(j * P, (j + 1) * P)
                                nc.vector.tensor_tensor(
                                    ctxT[bP * HPC + mP][
                                        :, qcP * NCH + j * P:
                                        qcP * NCH + (j + 1) * P],
                                    ctx_ps[:, sl], rec[:, sl], MUL,
                                )
                        else:
                            nc.vector.tensor_tensor(
                                ctxT[bP * HPC + mP][:, qcP * NCH:(qcP + 1) * NCH],
                                ctx_ps[:], rec[:], MUL,
                            )
                        if mP == HPC - 1:
                            oproj_q.extend(
                                (bP, qcP * VST + j) for j in range(VST))
                    return ex_new

                # m innermost so each (b, qc)'s output rows unblock every
                # other iteration -> oproj work and y DMAs spread evenly
                work = [(b, m, qc) for b in range(B) for qc in range(SQC)
                        for m in range(HPC)]
                ex_prev = None
                oproj_q = []
                for i in range(len(work) + 1):
                    cur = work[i] if i < len(work) else None
                    prev = work[i - 1] if i > 0 else None
                    ex_prev = fused_iter(cur, prev, ex_prev, oproj_q)
                while oproj_q:
                    oproj_block(*oproj_q.pop(0), tail=True)
    nc.finalize()
    return nc


def _rope_tables():
    inv_freq = (1.0 / (ROPE_BASE ** (np.arange(0, HD, 2, dtype=np.float32) / HD))).astype(np.float32)
    t = np.arange(S, dtype=np.float32)
    freqs = np.outer(t, inv_freq).astype(np.float32)  # [S, HD/2]
    c = np.cos(freqs).astype(np.float32).T            # [64, S]
    s = np.sin(freqs).astype(np.float32).T
    cos2 = np.concatenate([c, c], axis=0)             # [128, S]
    sin2 = np.concatenate([s, -s], axis=0)            # [128, S]: [s; -s]
    return np.ascontiguousarray(cos2), np.ascontiguousarray(sin2)


def kernel(x, Wq, Wk, Wv, Wo):
    from concourse.bass_utils import run_bass_kernel_spmd

    F16 = np.float16
    NQC = TOK // NCH

    x = np.asarray(x, dtype=np.float32)
    Wq = np.asarray(Wq, dtype=np.float32)
    Wk = np.asarray(Wk, dtype=np.float32)
    Wv = np.asarray(Wv, dtype=np.float32)
    Wo = np.asarray(Wo, dtype=np.float32)

    # x rearranged to [p, chunk, kt, token]: partition-contiguous DMA runs
    xR = np.ascontiguousarray(
        x.reshape(NQC, NCH, KT, P).transpose(3, 0, 2, 1).astype(F16)
    )
    cos2, sin2 = _rope_tables()
    cos2 = np.ascontiguousarray(cos2.astype(F16))
    sin2 = np.ascontiguousarray(sin2.astype(F16))
    ones128 = np.ones((P, P), dtype=F16)

    def wslices(W, c):  # [D, CPC] -> [p, kt, cpc]
        ch0, ch1 = c * CPC, (c + 1) * CPC
        wT = W[ch0:ch1, :].T  # [D, CPC]
        return np.ascontiguousarray(
            wT.reshape(KT, P, CPC).transpose(1, 0, 2).astype(F16)
        )

    in_maps = []
    for c in range(NCORES):
        ch0, ch1 = c * CPC, (c + 1) * CPC
        woT = Wo[:, ch0:ch1].T  # [CPC, D]
        in_maps.append({
            "xR": xR,
            "wqR": wslices(Wq, c),
            "wkR": wslices(Wk, c),
            "wvR": wslices(Wv, c),
            "woR": np.ascontiguousarray(
                woT.reshape(HPC, P, D).transpose(1, 0, 2).astype(F16)
            ),
            "cos2": cos2,
            "sin2": sin2,
            "ones128": ones128,
        })

    if "nc" not in _cache:
        _cache["nc"] = _build_nc()
    res = run_bass_kernel_spmd(_cache["nc"], in_maps, core_ids=list(range(NCORES)))
    _cache["last_results"] = res

    y = np.zeros((TOK, D), dtype=np.float32)
    for rm in res.results:
        y += rm["y"].astype(np.float32)
    return y.reshape(B, S, D)


# revision 40
# speedup vs baseline: 1.5730x; 1.0096x over previous
"""MultiHeadAttention (B=2, S=2048, D=2048, H=16, RoPE) on 8 NeuronCores.

Sharding: tensor-parallel over heads. Core c owns heads 2c, 2c+1 (256 channels).
Each core: QKV projections for its channels, RoPE, full attention for its 2
heads, and a partial output projection y_c = ctx_c @ Wo[:, ch_c].T. Host sums
the 8 partials (fp16 partials, fp32 sum).

All-fp16 compute with fp32 PSUM accumulation:
  - Host pre-rearranges x and weights into partition-major layouts so every
    DMA moves partition-contiguous runs.
  - Startup: the critical tiles (x chunk 0, wv, wq/wk) are DMAed in
    kt-quarter pieces spread across BOTH rings, interleaved in first-use
    order, so the first v-sweep matmul issues ~10us in instead of ~20us
    and chunk 0 completes ~11us earlier; ~40 warmup matmuls bridge the
    preamble and warm the PE HAM clock gate.
  - Projections: per 512-token chunk, v-sweep (stationary x subtiles) then
    qk-sweep (stationary weight tiles). PSUM: q 2 + k 2 banks single-buffered
    + v 4 banks. x streams on alternating sync/gpsimd rings.
  - Attention (transposed scores): software pipeline at kt-pair granularity:
    scores+exp for item i interleave with PV matmuls for item i-1. exp runs
    on 1024-element 2-bank ACTIVATEs. The softmax denominator is built by a
    log2 reduction tree over the 16 exp tiles (DVE does the two big levels,
    the idle GpSimd engine the two small ones), leaving a single ones[128,128]
    matmul to do the partition-reduce+broadcast; its PSUM tile comes from the
    same pool as the output-projection accumulators, which frees a bank so
    ctx_ps is double-buffered: the normalize chain (den mm -> reciprocal ->
    mul) no longer blocks the next item's PV matmuls.
  - Work is ordered (b, qc, m) with m innermost so output-projection row
    blocks become available every other iteration; each iteration emits up
    to two 128-row blocks mid/late in its kt loop (their normalize chain
    finishes ~8us after queueing), keeping the PE ahead of the exp-bound
    part of the pipeline and spreading y writes evenly across the sync and
    gpsimd rings. Once scores are done, drain blocks borrow the dead score
    PSUM pool (2-bank tiles, one eviction per pair split ACT/DVE) so the
    drain is matmul-paced, each pair's half-row y DMA issued right behind
    its eviction on alternating rings; the last item's denominator tree
    stays on DVE and its normalize is split per row-block to start the
    drain sooner.
"""
import sys

sys.path.insert(0, "/opt/trn_rl_repo")

import numpy as np

B, S, D, H = 2, 2048, 2048, 16
HD = D // H          # 128
NCORES = 8
HPC = H // NCORES    # heads per core
CPC = HPC * HD       # channels per core = 256
TOK = B * S          # 4096
P = 128
KT = D // P          # 16 contraction tiles
NCH = 512            # token chunk for projections / attention qtok chunk
ROPE_BASE = 10000.0

_cache = {}


def _build_nc():
    import concourse.bass as bass  # noqa: F401
    import concourse.mybir as mybir
    import concourse.tile as tile
    from concourse import bacc

    F32 = mybir.dt.float32
    F16 = mybir.dt.float16
    AF = mybir.ActivationFunctionType
    MUL = mybir.AluOpType.mult
    ADD = mybir.AluOpType.add

    nc = bacc.Bacc(None, target_bir_lowering=False)

    NQC = TOK // NCH            # 8 projection chunks
    SQC = S // NCH              # 4 attention q-chunks per sequence
    SKT = S // P                # 16 key tiles per sequence
    VST = NCH // P              # 4 v subtiles per chunk
    QT = KT // 4                # 4 kt per x quarter tile
    HF = HD // 2                # 64
    SCALE = 1.0 / float(np.sqrt(HD))

    # host-rearranged inputs: partition-major, contiguous per partition
    xT_d = nc.dram_tensor("xR", [P, NQC, KT, NCH], F16, kind="ExternalInput")
    wq_d = nc.dram_tensor("wqR", [P, KT, CPC], F16, kind="ExternalInput")
    wk_d = nc.dram_tensor("wkR", [P, KT, CPC], F16, kind="ExternalInput")
    wv_d = nc.dram_tensor("wvR", [P, KT, CPC], F16, kind="ExternalInput")
    wo_d = nc.dram_tensor("woR", [P, HPC, D], F16, kind="ExternalInput")
    cos_d = nc.dram_tensor("cos2", [P, S], F16, kind="ExternalInput")
    sin_d = nc.dram_tensor("sin2", [P, S], F16, kind="ExternalInput")
    ones_d = nc.dram_tensor("ones128", [P, P], F16, kind="ExternalInput")
    y_d = nc.dram_tensor("y", [TOK, D], F16, kind="ExternalOutput")

    with tile.TileContext(nc) as tc, \
         nc.allow_low_precision(reason="fp16 compute, fp32 accumulate"):
        with tc.tile_pool(name="persist", bufs=1) as pp_:
            # long-lived tensors
            qT = [pp_.tile([P, TOK], F16, name=f"qT{m}") for m in range(HPC)]
            kTt = [pp_.tile([P, TOK], F16, name=f"kT{m}") for m in range(HPC)]
            vS = pp_.tile([P, TOK // P, CPC], F16, name="vS")

            wq = pp_.tile([P, KT, CPC], F16, name="wq")
            wk = pp_.tile([P, KT, CPC], F16, name="wk")
            wv = pp_.tile([P, KT, CPC], F16, name="wv")
            cos2 = pp_.tile([P, S], F16, name="cos2")
            sin2 = pp_.tile([P, S], F16, name="sin2")
            wo = pp_.tile([P, HPC, D], F16, name="wo")
            ones128 = pp_.tile([P, P], F16, name="ones128")
            def _wv(qq):
                nc.gpsimd.dma_start(
                    wv[:, qq * QT:(qq + 1) * QT, :],
                    wv_d[:, qq * QT:(qq + 1) * QT, :],
                )

            # warm the exp table set so ACT_TABLE_LOAD is off the critical path
            warm_in = pp_.tile([1, 1], F32, name="warm_in")
            warm = pp_.tile([1, 1], F32, name="warm")
            nc.vector.memset(warm_in[:], 0.0)
            nc.scalar.activation(warm[:], warm_in[:], AF.Exp)

            # a short run of dummy matmuls on a zeroed tile bridges the ~2us
            # between the preamble and the first x/wv pieces landing, and
            # starts the PE HAM warm-up window early
            wt16 = pp_.tile([P, P], F16, name="wt16")
            nc.vector.memset(wt16[:], 0.0)
            with tc.tile_pool(name="wrm", bufs=1, space="PSUM") as wrmp:
                wrm_ps = wrmp.tile([P, P], F32, name="wrm_ps")
                for _ in range(40):
                    nc.tensor.matmul(wrm_ps[:], wt16[:], wt16[:],
                                     start=True, stop=True)

            # ---------------- Phase 1: projections + RoPE ----------------
            with tc.tile_pool(name="xp", bufs=12) as xp, \
                 tc.tile_pool(name="stg", bufs=4) as stg, \
                 tc.tile_pool(name="rp", bufs=4) as rp, \
                 tc.tile_pool(name="qkp", bufs=1, space="PSUM") as qkp, \
                 tc.tile_pool(name="vp", bufs=1, space="PSUM") as vp:
                for ch in range(NQC):
                    t0 = ch * NCH
                    s0 = (ch % SQC) * NCH  # position within sequence
                    # x for this chunk: [128, 16, 512] in four kt-quarter
                    # DMAs so chunk 0's first pieces land early; rings
                    # alternate per chunk so each carries ~half the stream
                    xt = [xp.tile([P, QT, NCH], F16, name="xt")
                          for _ in range(4)]
                    eng = nc.gpsimd if ch % 2 else nc.sync
                    if ch == 0:
                        # chunk 0 is the only DMA-paced chunk: spread its x
                        # quarters across BOTH rings, interleaved with the wv
                        # and wq/wk pieces in first-use order so neither ring
                        # serializes more than ~1MB ahead of the sweeps
                        def _x(qq, e):
                            e.dma_start(
                                xt[qq][:], xT_d[:, ch, qq * QT:(qq + 1) * QT, :]
                            )
                        def _w(qq):
                            nc.sync.dma_start(
                                wq[:, qq * QT:(qq + 1) * QT, :],
                                wq_d[:, qq * QT:(qq + 1) * QT, :],
                            )
                            nc.sync.dma_start(
                                wk[:, qq * QT:(qq + 1) * QT, :],
                                wk_d[:, qq * QT:(qq + 1) * QT, :],
                            )
                        _x(0, nc.sync)
                        _wv(0)
                        _x(1, nc.gpsimd)
                        _x(2, nc.sync)
                        _wv(1)
                        _w(0)
                        _wv(2)
                        _x(3, nc.gpsimd)
                        _w(1)
                        _wv(3)
                        _w(2)
                        _w(3)
                        nc.sync.dma_start(cos2[:], cos_d[:])
                        nc.sync.dma_start(sin2[:], sin_d[:])
                    else:
                        for qq in range(4):
                            eng.dma_start(
                                xt[qq][:], xT_d[:, ch, qq * QT:(qq + 1) * QT, :]
                            )
                    if ch == 1:
                        # needed only in phase 2; behind chunk 1 on gpsimd
                        nc.gpsimd.dma_start(wo[:], wo_d[:])
                        nc.gpsimd.dma_start(ones128[:], ones_d[:])

                    # ---- v-sweep: stationary x subtiles, moving wv ----
                    v_ps = vp.tile([P, VST, 512], F32, name="v_ps")
                    for kt in range(KT):
                        xtile = xt[kt // QT][:, kt % QT, :]
                        st_, sp_ = (kt == 0), (kt == KT - 1)
                        for st in range(VST):
                            nc.tensor.matmul(
                                v_ps[:, st, 0:CPC],
                                xtile[:, st * P:(st + 1) * P],
                                wv[:, kt, :],
                                start=st_, stop=sp_,
                            )
                    # evict v on ScalarE (fp32 psum -> fp16 sbuf)
                    nc.scalar.activation(
                        vS[:, ch * VST:(ch + 1) * VST, :], v_ps[:, :, 0:CPC],
                        AF.Copy,
                    )

                    # ---- qk-sweep: stationary weights, moving x ----
                    q_ps = qkp.tile([P, HPC, NCH], F32, name="q_ps")
                    k_ps = qkp.tile([P, HPC, NCH], F32, name="k_ps")
                    for kt in range(KT):
                        xtile = xt[kt // QT][:, kt % QT, :]
                        st_, sp_ = (kt == 0), (kt == KT - 1)
                        for m in range(HPC):
                            nc.tensor.matmul(
                                q_ps[:, m, :], wq[:, kt, m * P:(m + 1) * P],
                                xtile[:], start=st_, stop=sp_,
                            )
                            nc.tensor.matmul(
                                k_ps[:, m, :], wk[:, kt, m * P:(m + 1) * P],
                                xtile[:], start=st_, stop=sp_,
                            )
                    # stage q/k to SBUF fp16 on ScalarE (frees PSUM fast)
                    qsb = stg.tile([P, HPC, NCH], F16, name="qsb")
                    ksb = stg.tile([P, HPC, NCH], F16, name="ksb")
                    nc.scalar.activation(qsb[:], q_ps[:], AF.Copy)
                    nc.scalar.activation(ksb[:], k_ps[:], AF.Copy)

                    # RoPE on VectorE in fp16: out = src*cos2 + swap(src)*sin2
                    # sin2 is laid out [s; -s] so each half-product reads its
                    # inputs at a shared base partition (DVE requirement) and
                    # only the OUTPUT lands in the opposite half:
                    #   rot[0:64]   = src[64:128]*sin2[64:128]  (= -s half)
                    #   rot[64:128] = src[0:64]  *sin2[0:64]    (= +s half)
                    for m in range(HPC):
                        for src, dst in ((qsb, qT[m]), (ksb, kTt[m])):
                            sm = src[:, m, :]
                            rot = rp.tile([P, NCH], F16, name="rot")
                            nc.vector.tensor_tensor(
                                rot[0:HF, :], sm[HF:P, :],
                                sin2[HF:P, s0:s0 + NCH], MUL,
                            )
                            nc.vector.tensor_tensor(
                                rot[HF:P, :], sm[0:HF, :],
                                sin2[0:HF, s0:s0 + NCH], MUL,
                            )
                            tmp = rp.tile([P, NCH], F16, name="tmp")
                            nc.vector.tensor_tensor(
                                tmp[:], sm[:], cos2[:, s0:s0 + NCH], MUL
                            )
                            nc.vector.tensor_tensor(
                                dst[:, t0:t0 + NCH], tmp[:], rot[:], ADD
                            )

            # ---------------- Phase 2+3: attention + output projection ----
            with tc.tile_pool(name="ep", bufs=2) as ep, \
                 tc.tile_pool(name="tp", bufs=2) as tp, \
                 tc.tile_pool(name="dp", bufs=2) as dp, \
                 tc.tile_pool(name="yp", bufs=4) as yp, \
                 tc.tile_pool(name="ctxp", bufs=1) as ctxp, \
                 tc.tile_pool(name="cp", bufs=2, space="PSUM") as cpsum, \
                 tc.tile_pool(name="ap", bufs=2, space="PSUM") as apsum, \
                 tc.tile_pool(name="sp2", bufs=2, space="PSUM") as spsum:
                ctxT = [
                    ctxp.tile([P, S], F16, name=f"ctxT{b}_{m}")
                    for b in range(B)
                    for m in range(HPC)
                ]

                nblk = [0]  # global oproj block counter (ring + evict split)

                def oproj_block(b, tt, tail=False):
                    # one 128-row block of y: 4 nck groups x 2 heads
                    row0 = b * S + tt * P
                    i = nblk[0]
                    nblk[0] += 1
                    y_sb = yp.tile([P, D], F16, name="y_sb")
                    if tail:
                        # scores are finished: borrow the (dead) scr PSUM
                        # pool -- 2-bank tiles, one eviction per pair and a
                        # half-row DMA right behind it, so the drain is
                        # matmul-paced and the HBM writes start ASAP
                        for pr in range(2):
                            y2 = spsum.tile([P, 2, NCH], F32, name="scr")
                            for j in range(2):
                                nck = pr * 2 + j
                                for m in range(HPC):
                                    nc.tensor.matmul(
                                        y2[:, j, :],
                                        ctxT[b * HPC + m][:, tt * P:(tt + 1) * P],
                                        wo[:, m, nck * NCH:(nck + 1) * NCH],
                                        start=(m == 0), stop=(m == HPC - 1),
                                    )
                            half = y_sb[:, pr * (D // 2):(pr + 1) * (D // 2)]
                            if pr == 0:
                                nc.scalar.activation(half, y2[:], AF.Copy)
                            else:
                                nc.vector.tensor_copy(half, y2[:])
                            eng = nc.sync if (i + pr) % 2 == 0 else nc.gpsimd
                            eng.dma_start(
                                y_d[row0:row0 + P,
                                    pr * (D // 2):(pr + 1) * (D // 2)],
                                half,
                            )
                        return
                    for nck in range(D // NCH):
                        y_ps = apsum.tile([P, NCH], F32, name="y_ps")
                        for m in range(HPC):
                            nc.tensor.matmul(
                                y_ps[:],
                                ctxT[b * HPC + m][:, tt * P:(tt + 1) * P],
                                wo[:, m, nck * NCH:(nck + 1) * NCH],
                                start=(m == 0), stop=(m == HPC - 1),
                            )
                        # ~1 in 8 evictions on ScalarE (it is exp-bound),
                        # rest on VectorE
                        if (i * 4 + nck) % 8 == 3:
                            nc.scalar.activation(
                                y_sb[:, nck * NCH:(nck + 1) * NCH],
                                y_ps[:], AF.Copy,
                            )
                        else:
                            nc.vector.tensor_copy(
                                y_sb[:, nck * NCH:(nck + 1) * NCH], y_ps[:],
                            )
                    eng = nc.sync if i % 2 == 0 else nc.gpsimd
                    eng.dma_start(y_d[row0:row0 + P, :], y_sb[:])

                def fused_iter(cur, prev, ex_prev, oproj_q):
                    """Interleave scores+exp for `cur` with PV matmuls for
                    `prev` at kt-pair granularity; emit up to two queued
                    output-projection blocks per iteration."""
                    blks = [oproj_q.pop(0) for _ in
                            range(min(2, len(oproj_q)))]
                    ex_new = None
                    if cur is not None:
                        bC, mC, qcC = cur
                        qt0 = bC * S + qcC * NCH
                        ex_new = ep.tile([P, SKT, NCH], F16, name="ex")
                    if prev is not None:
                        bP, mP, qcP = prev
                        ctx_ps = cpsum.tile([P, NCH], F32, name="ctx_ps")
                        # denominator reduction tree over ex_prev's 16 tiles:
                        # two big levels on DVE, two small ones on GpSimd
                        # (idle in this phase); a single ones-matmul then
                        # partition-reduces + broadcasts
                        t8 = tp.tile([P, 8, NCH], F16, name="t8")
                        u4 = tp.tile([P, 4, NCH], F16, name="u4")
                        es = tp.tile([P, 2, NCH], F16, name="es")
                        nc.vector.tensor_tensor(
                            t8[:], ex_prev[:, 0:8, :], ex_prev[:, 8:16, :],
                            ADD,
                        )
                        nc.vector.tensor_tensor(
                            u4[:], t8[:, 0:4, :], t8[:, 4:8, :], ADD,
                        )
                        # the small tree levels go to the idle GpSimd engine
                        # (relieves DVE), except for the final item where the
                        # extra cross-engine latency would push the drain out
                        tree_eng = nc.vector if cur is None else nc.gpsimd
                        tree_eng.tensor_tensor(
                            es[:], u4[:, 0:2, :], u4[:, 2:4, :], ADD,
                        )
                        tree_eng.tensor_tensor(
                            es[:, 0, :], es[:, 0, :], es[:, 1, :], ADD,
                        )
                    for kp in range(SKT // 2):
                        # oproj blocks sit mid/late in the iteration: their
                        # (b, qc) normalize chain (tree -> den -> recip ->
                        # mul) completes ~8us after the queueing iteration
                        # starts, so an iteration-start emission would stall
                        if kp == 4 and blks:
                            oproj_block(*blks[0], tail=(cur is None))
                        if kp == 7 and len(blks) > 1:
                            oproj_block(*blks[1], tail=(cur is None))
                        if cur is not None:
                            scr = spsum.tile([P, 2, NCH], F32, name="scr")
                            for j in range(2):
                                kt = kp * 2 + j
                                nc.tensor.matmul(
                                    scr[:, j, :],
                                    kTt[mC][:, bC * S + kt * P:
                                             bC * S + (kt + 1) * P],
                                    qT[mC][:, qt0:qt0 + NCH],
                                    start=True, stop=True,
                                )
                            nc.scalar.activation(
                                ex_new[:, kp * 2:kp * 2 + 2, :], scr[:],
                                AF.Exp, scale=SCALE,
                            )
                        if prev is not None:
                            for j in range(2):
                                kt = kp * 2 + j
                                gkt = bP * SKT + kt
                                st_, sp_ = (kt == 0), (kt == SKT - 1)
                                nc.tensor.matmul(
                                    ctx_ps[:],
                                    vS[:, gkt, mP * P:(mP + 1) * P],
                                    ex_prev[:, kt, :],
                                    start=st_, stop=sp_,
                                )

                    if prev is not None:
                        # den_ps shares the oproj PSUM pool (frees a bank so
                        # ctx_ps is double-buffered); every row of the
                        # full-stationary accumulator IS the denominator
                        den_ps = apsum.tile([P, NCH], F32, name="y_ps")
                        nc.tensor.matmul(
                            den_ps[:], ones128[:], es[:, 0, :],
                            start=True, stop=True,
                        )
                        rec = dp.tile([P, NCH], F32, name="rec")
                        nc.vector.reciprocal_approx_fast(out=rec[:], in_=den_ps[:])
                        if cur is None:
                            # final item: normalize per row-block so each
                            # drain oproj block starts behind its own slice
                            for j in range(VST):
                                sl = slice(j * P, (j + 1) * P)
                                nc.vector.tensor_tensor(
                                    ctxT[bP * HPC + mP][
                                        :, qcP * NCH + j * P:
                                        qcP * NCH + (j + 1) * P],
                                    ctx_ps[:, sl], rec[:, sl], MUL,
                                )
                        else:
                            nc.vector.tensor_tensor(
                                ctxT[bP * HPC + mP][:, qcP * NCH:(qcP + 1) * NCH],
                                ctx_ps[:], rec[:], MUL,
                            )
                        if mP == HPC - 1:
                            oproj_q.extend(
                                (bP, qcP * VST + j) for j in range(VST))
                    return ex_new

                # m innermost so each (b, qc)'s output rows unblock every
                # other iteration -> oproj work and y DMAs spread evenly
                work = [(b, m, qc) for b in range(B) for qc in range(SQC)
                        for m in range(HPC)]
                ex_prev = None
                oproj_q = []
                for i in range(len(work) + 1):
                    cur = work[i] if i < len(work) else None
                    prev = work[i - 1] if i > 0 else None
                    ex_prev = fused_iter(cur, prev, ex_prev, oproj_q)
                while oproj_q:
                    oproj_block(*oproj_q.pop(0), tail=True)
    nc.finalize()
    return nc


def _rope_tables():
    inv_freq = (1.0 / (ROPE_BASE ** (np.arange(0, HD, 2, dtype=np.float32) / HD))).astype(np.float32)
    t = np.arange(S, dtype=np.float32)
    freqs = np.outer(t, inv_freq).astype(np.float32)  # [S, HD/2]
    c = np.cos(freqs).astype(np.float32).T            # [64, S]
    s = np.sin(freqs).astype(np.float32).T
    cos2 = np.concatenate([c, c], axis=0)             # [128, S]
    sin2 = np.concatenate([s, -s], axis=0)            # [128, S]: [s; -s]
    return np.ascontiguousarray(cos2), np.ascontiguousarray(sin2)


def kernel(x, Wq, Wk, Wv, Wo):
    from concourse.bass_utils import run_bass_kernel_spmd

    F16 = np.float16
    NQC = TOK // NCH

    x = np.asarray(x, dtype=np.float32)
    Wq = np.asarray(Wq, dtype=np.float32)
    Wk = np.asarray(Wk, dtype=np.float32)
    Wv = np.asarray(Wv, dtype=np.float32)
    Wo = np.asarray(Wo, dtype=np.float32)

    # x rearranged to [p, chunk, kt, token]: partition-contiguous DMA runs
    xR = np.ascontiguousarray(
        x.reshape(NQC, NCH, KT, P).transpose(3, 0, 2, 1).astype(F16)
    )
    cos2, sin2 = _rope_tables()
    cos2 = np.ascontiguousarray(cos2.astype(F16))
    sin2 = np.ascontiguousarray(sin2.astype(F16))
    ones128 = np.ones((P, P), dtype=F16)

    def wslices(W, c):  # [D, CPC] -> [p, kt, cpc]
        ch0, ch1 = c * CPC, (c + 1) * CPC
        wT = W[ch0:ch1, :].T  # [D, CPC]
        return np.ascontiguousarray(
            wT.reshape(KT, P, CPC).transpose(1, 0, 2).astype(F16)
        )

    in_maps = []
    for c in range(NCORES):
        ch0, ch1 = c * CPC, (c + 1) * CPC
        woT = Wo[:, ch0:ch1].T  # [CPC, D]
        in_maps.append({
            "xR": xR,
            "wqR": wslices(Wq, c),
            "wkR": wslices(Wk, c),
            "wvR": wslices(Wv, c),
            "woR": np.ascontiguousarray(
                woT.reshape(HPC, P, D).transpose(1, 0, 2).astype(F16)
            ),
            "cos2": cos2,
            "sin2": sin2,
            "ones128": ones128,
        })

    if "nc" not in _cache:
        _cache["nc"] = _build_nc()
    res = run_bass_kernel_spmd(_cache["nc"], in_maps, core_ids=list(range(NCORES)))
    _cache["last_results"] = res

    y = np.zeros((TOK, D), dtype=np.float32)
    for rm in res.results:
        y += rm["y"].astype(np.float32)
    return y.reshape(B, S, D)
